# revision 1
# baseline (speedup 1.0000x reference)
"""Tensor-parallel GQA multi-head attention (RoPE + causal softmax) for 8 trn2 cores.

Sharding: 8 cores = 2 batches x 4 head-groups. Core c handles batch c//4 and
q-heads [8g, 8g+8) / kv-heads {2g, 2g+1} where g = c%4. Each core projects its
batch's tokens with its weight shard, runs flash-style causal attention in
transposed (feature-major) layout, applies the output projection, and the four
cores of a batch AllReduce the partial [S, D] output. Host stacks batch 0/1.
"""

import sys

sys.path.insert(0, "/opt/trn_rl_repo")

import numpy as np

import concourse.bass as bass
import concourse.bacc as bacc
import concourse.mybir as mybir
from concourse import tile
from concourse.bass_utils import run_bass_kernel_spmd

B, S, D = 2, 2048, 2048
N_HEADS, N_KV, HD = 32, 8, 64
NCORES = 8
NG = 4  # head groups = cores per batch
QH = 8  # q-heads per core
KVH = 2  # kv-heads per core
FQ = QH * HD  # 512
FKV = 2 * KVH * HD  # 256 (K then V)
SCALE = 1.0 / 8.0  # 1/sqrt(HD)
MASK_NEG = -30000.0

QTILE = 512
KTILE = 128
NSLAB = S // QTILE  # 4
ND = D // 128  # 16 contraction chunks
NKT = S // KTILE  # 16

F32 = mybir.dt.float32
F32R = mybir.dt.float32r
EXP = mybir.ActivationFunctionType.Exp

BF16 = mybir.dt.bfloat16
# matmul operand dtype
MMD = BF16


def _c(ap):
    return ap  # operands are float32r-native


def _build_kernel(tc, io):
    nc = tc.nc
    xT, wq, wkv, wo = io["xT"], io["wq"], io["wkv"], io["wo"]
    cos2, sin2s, trimask, sel = io["cos2"], io["sin2s"], io["trimask"], io["sel"]
    out_full = io["out"]
    single = bool(io.get("single"))

    # ---- pools (LIFO per side; SBUF per-partition budget is tight) ----
    const = tc.alloc_tile_pool(name="const", bufs=1)
    dram = tc.alloc_tile_pool(name="dram", bufs=1, space="DRAM")
    qkv = tc.alloc_tile_pool(name="qkv", bufs=1)              # QT, KK
    vap = tc.alloc_tile_pool(name="vap", bufs=1)              # V chunks

    trimask_t = const.tile([KTILE, KTILE], F32)
    nc.sync.dma_start(trimask_t[:], trimask[:])
    sel_t = const.tile([QH, FQ], MMD)
    nc.sync.dma_start(sel_t[:], sel[:])
    ident = const.tile([128, 64], F32)
    nc.gpsimd.memset(ident[:], 0.0)
    for p in (0, 64):
        nc.gpsimd.affine_select(
            out=ident[p:p + 64, :], in_=ident[p:p + 64, :],
            compare_op=mybir.AluOpType.not_equal,
            fill=1.0, base=0, pattern=[[-1, 64]], channel_multiplier=1,
        )
    ones_col = const.tile([128, 1], F32)
    nc.vector.memset(ones_col[:], 1.0)

    QT = [qkv.tile([128, S], MMD, name=f"qt{t}") for t in range(4)]
    KK = qkv.tile([128, S], MMD)  # rows 0:64 K^T kv0, 64:128 K^T kv1

    # ============ phase A: projections + RoPE + V transpose, per slab ======
    tables = tc.alloc_tile_pool(name="tables", bufs=1)
    cos2_t = tables.tile([128, S], MMD)
    nc.sync.dma_start(cos2_t[:], cos2[:])
    sin2s_t = tables.tile([128, S], MMD)
    nc.sync.dma_start(sin2s_t[:], sin2s[:])

    wA = tc.alloc_tile_pool(name="wA", bufs=1)
    xq_pool = tc.alloc_tile_pool(name="xq", bufs=1)
    rp = tc.alloc_tile_pool(name="rope", bufs=2)
    psA = tc.alloc_tile_pool(name="psA", bufs=4, space="PSUM")
    psC = tc.alloc_tile_pool(name="psC", bufs=2, space="PSUM")

    def load_xslab(j):
        xts = []
        for k in range(ND):
            xt = xq_pool.tile([128, QTILE], MMD, name="xt", tag=f"xt{k}")
            nc.sync.dma_start(
                xt[:], xT[k * 128:(k + 1) * 128, j * QTILE:(j + 1) * QTILE])
            xts.append(xt)
        return xts

    xts = load_xslab(0)  # first activations slab before the weight bulk
    Wt = {}
    for f in range(6):
        for k in range(ND):
            w = wA.tile([128, 128], MMD, name=f"w{f}_{k}")
            if f < 4:
                wsrc = wq[k * 128:(k + 1) * 128, f * 128:(f + 1) * 128]
            else:
                wsrc = wkv[k * 128:(k + 1) * 128, (f - 4) * 128:(f - 3) * 128]
            nc.sync.dma_start(w[:], wsrc)
            Wt[f, k] = w

    VA = {}
    for j in range(NSLAB):
        qs = slice(j * QTILE, (j + 1) * QTILE)
        if j > 0:
            xts = load_xslab(j)
        for f in range(6):
            ps = psA.tile([128, QTILE], F32, name="psA", tag="psA")
            for k in range(ND):
                nc.tensor.matmul(ps[:], Wt[f, k][:], xts[k][:],
                                 start=(k == 0), stop=(k == ND - 1))
            if f < 5:
                # QT tiles 0..3 and KK: evacuate then RoPE the slab in place
                dst = QT[f] if f < 4 else KK
                nc.scalar.copy(dst[:, qs], ps[:])
                qsw = rp.tile([128, QTILE], MMD, name="qsw", tag="qsw")
                for p in (0, 64):
                    nc.sync.dma_start(qsw[p:p + 32, :], dst[p + 32:p + 64, qs])
                    nc.sync.dma_start(qsw[p + 32:p + 64, :], dst[p:p + 32, qs])
                t1 = rp.tile([128, QTILE], F32, name="t1", tag="t1")
                nc.vector.tensor_mul(t1[:], dst[:, qs], cos2_t[:, qs])
                t2 = rp.tile([128, QTILE], F32, name="t2", tag="t2")
                nc.vector.tensor_mul(t2[:], qsw[:], sin2s_t[:, qs])
                nc.vector.tensor_add(dst[:, qs], t1[:], t2[:])
            else:
                # V^T slab: evacuate then transpose 128-chunks into [k, d]
                vv = rp.tile([128, QTILE], F32, name="vv", tag="vv")
                nc.scalar.copy(vv[:], ps[:])
                for kv in range(KVH):
                    for c in range(4):
                        i = 4 * j + c
                        tp = psC.tile([128, HD], F32, name="tp", tag="tp")
                        nc.tensor.matmul(tp[:], vv[kv * 64:(kv + 1) * 64,
                                                    c * 128:(c + 1) * 128],
                                         ident[kv * 64:(kv + 1) * 64, :],
                                         is_transpose=True, start=True,
                                         stop=True)
                        va = vap.tile([128, HD + 1], MMD, name=f"va{kv}_{i}")
                        nc.scalar.copy(va[:, 0:HD], tp[:])
                        nc.scalar.copy(va[:, HD:HD + 1], ones_col[:])
                        VA[kv, i] = va
    psC.release()
    psA.release()
    rp.release()
    xq_pool.release()
    wA.release()
    tables.release()

    # ============ attention + normalize + wo + reduce-scatter, pipelined ===
    wop = tc.alloc_tile_pool(name="wop", bufs=1, side="right")
    WO = {}
    for fc in range(4):
        for dn in range(4):
            w = wop.tile([128, QTILE], MMD, name=f"wo{fc}_{dn}")
            nc.sync.dma_start(
                w[:], wo[fc * 128:(fc + 1) * 128, dn * QTILE:(dn + 1) * QTILE])
            WO[fc, dn] = w
    aop = tc.alloc_tile_pool(name="aop", bufs=1, side="right")
    AO = [aop.tile([128, S], MMD, name=f"ao{t}") for t in range(4)]
    denom = aop.tile([QH, S], MMD)
    denomR = aop.tile([QH, S], F32)
    denomRb = aop.tile([QH, S], MMD)

    partial = dram.tile([S, D], F32)
    rs_out = [dram.tile([64, D], F32, name=f"rs{h}") for h in range(2 * NSLAB)]

    psS = tc.alloc_tile_pool(name="psS", bufs=2, space="PSUM")
    psW = tc.alloc_tile_pool(name="psW", bufs=2, space="PSUM")
    pexp = tc.alloc_tile_pool(name="pexp", bufs=3)
    evac = tc.alloc_tile_pool(name="evac", bufs=2)

    def attention_slab(j):
        qs = slice(j * QTILE, (j + 1) * QTILE)
        for t in range(4):
            oA = psS.tile([HD + 1, QTILE], F32, name="oA", tag="o")
            oB = psS.tile([HD + 1, QTILE], F32, name="oB", tag="o")
            nkt = 4 * j + 4
            for i in range(nkt):
                r = i - 4 * j
                off = max(r, 0) * KTILE
                ks = slice(i * KTILE, (i + 1) * KTILE)
                qv = slice(j * QTILE + off, (j + 1) * QTILE)
                sA = psS.tile([KTILE, QTILE], F32, name="sA", tag="sA")
                sB = psS.tile([KTILE, QTILE], F32, name="sB", tag="sB")
                nc.tensor.matmul(sA[:, off:], KK[0:64, ks], QT[t][0:64, qv],
                                 start=True, stop=True, tile_position=(0, 0))
                nc.tensor.matmul(sB[:, off:], KK[64:128, ks], QT[t][64:128, qv],
                                 start=True, stop=True, tile_position=(64, 0))
                if r >= 0:
                    nc.vector.tensor_add(sA[:, off:off + KTILE],
                                         sA[:, off:off + KTILE], trimask_t[:])
                    nc.vector.tensor_add(sB[:, off:off + KTILE],
                                         sB[:, off:off + KTILE], trimask_t[:])
                pA = pexp.tile([KTILE, QTILE], MMD, name="pA", tag="pA")
                pB = pexp.tile([KTILE, QTILE], MMD, name="pB", tag="pB")
                nc.scalar.activation(pA[:, off:], sA[:, off:], EXP, scale=SCALE)
                nc.scalar.activation(pB[:, off:], sB[:, off:], EXP, scale=SCALE)
                nc.tensor.matmul(oA[:, off:], VA[0, i][:], pA[:, off:],
                                 start=(i == 0), stop=(i == nkt - 1))
                nc.tensor.matmul(oB[:, off:], VA[1, i][:], pB[:, off:],
                                 start=(i == 0), stop=(i == nkt - 1))
            # evacuate: rows 0:64 outT, row 64 denominator
            tA = evac.tile([HD + 1, QTILE], MMD, name="tA", tag="tA")
            tB = evac.tile([HD + 1, QTILE], MMD, name="tB", tag="tB")
            nc.vector.tensor_copy(tA[:], oA[:])
            nc.vector.tensor_copy(tB[:], oB[:])
            nc.sync.dma_start(AO[t][0:64, qs], tA[0:64, :])
            nc.sync.dma_start(AO[t][64:128, qs], tB[0:64, :])
            nc.sync.dma_start(denom[t:t + 1, qs], tA[64:65, :])
            nc.sync.dma_start(denom[t + 4:t + 5, qs], tB[64:65, :])

    def finish_slab(j):
        qs = slice(j * QTILE, (j + 1) * QTILE)
        nc.vector.reciprocal(denomR[:, qs], denom[:, qs])
        nc.vector.tensor_copy(denomRb[:, qs], denomR[:, qs])
        for t in range(4):
            bc = psW.tile([128, QTILE], F32, name="bc", tag="w")
            nc.tensor.matmul(bc[:], sel_t[:, t * 128:(t + 1) * 128],
                             denomRb[:, qs], start=True, stop=True)
            nc.vector.tensor_mul(AO[t][:, qs], AO[t][:, qs], bc[:])
        for jq in range(4 * j, 4 * j + 4):  # q-tiles of 128
            qsl = slice(jq * 128, (jq + 1) * 128)
            for dn in range(4):
                ps = psW.tile([128, QTILE], F32, name="psWo", tag="w")
                for fc in range(4):
                    nc.tensor.matmul(ps[:], AO[fc][:, qsl], WO[fc, dn][:],
                                     start=(fc == 0), stop=(fc == 3))
                og = evac.tile([128, QTILE], F32, name="og", tag="og")
                nc.vector.tensor_copy(og[:], ps[:])
                nc.sync.dma_start(
                    partial[jq * 128:(jq + 1) * 128,
                            dn * QTILE:(dn + 1) * QTILE], og[:])
        for h in (2 * j, 2 * j + 1):  # two half-slab reduce-scatters
            rows = slice(h * 256, (h + 1) * 256)
            if single:
                nc.sync.dma_start(rs_out[h][:], partial[h * 256:h * 256 + 64, :])
            else:
                nc.gpsimd.collective_compute(
                    "ReduceScatter",
                    mybir.AluOpType.add,
                    replica_groups=[[0, 1, 2, 3], [4, 5, 6, 7]],
                    ins=[partial[rows, :]],
                    outs=[rs_out[h][:]],
                )
            nc.sync.dma_start(out_full[h * 64:(h + 1) * 64, :], rs_out[h][:])

    for j in range(NSLAB):
        attention_slab(j)
        if j > 0:
            finish_slab(j - 1)
    finish_slab(NSLAB - 1)

    psW.release()
    psS.release()
    evac.release()
    pexp.release()
    aop.release()
    wop.release()
    vap.release()
    qkv.release()
    dram.release()
    const.release()


def _build(single=False):
    nc = bacc.Bacc("TRN2", target_bir_lowering=False, debug=False,
                   num_devices=1 if single else NCORES)
    io = {
        "xT": nc.dram_tensor("xT", [D, S], BF16, kind="ExternalInput").ap(),
        "wq": nc.dram_tensor("wq", [D, FQ], BF16, kind="ExternalInput").ap(),
        "wkv": nc.dram_tensor("wkv", [D, FKV], BF16, kind="ExternalInput").ap(),
        "wo": nc.dram_tensor("wo", [FQ, D], BF16, kind="ExternalInput").ap(),
        "cos2": nc.dram_tensor("cos2", [128, S], BF16, kind="ExternalInput").ap(),
        "sin2s": nc.dram_tensor("sin2s", [128, S], BF16, kind="ExternalInput").ap(),
        "trimask": nc.dram_tensor("trimask", [KTILE, KTILE], F32,
                                  kind="ExternalInput").ap(),
        "sel": nc.dram_tensor("sel", [QH, FQ], BF16, kind="ExternalInput").ap(),
        "out": nc.dram_tensor("out", [2 * NSLAB * 64, D], F32, kind="ExternalOutput").ap(),
    }
    io["single"] = single
    with tile.TileContext(nc) as tc:
        _build_kernel(tc, io)
    nc.compile()
    return nc


_CACHE = {}


def _get_program():
    if "nc" not in _CACHE:
        _CACHE["nc"] = _build()
    return _CACHE["nc"]


def _host_inputs(x, wq, wk, wv, wo):
    x = np.ascontiguousarray(x, np.float32)
    inv = 1.0 / (10000.0 ** (np.arange(0, HD, 2, dtype=np.float64) / HD))
    pos = np.arange(S, dtype=np.float64)
    freqs = np.outer(pos, inv)  # [S, 32]
    emb = np.concatenate([freqs, freqs], axis=1)  # [S, 64]
    cos = np.cos(emb).T.astype(np.float32)  # [64, S]
    sin = np.sin(emb).T.astype(np.float32)
    cos2 = np.concatenate([cos, cos], axis=0)  # [128, S]
    sin2s = np.concatenate([-sin[:32], sin[32:], -sin[:32], sin[32:]], axis=0)

    kk, qq = np.meshgrid(np.arange(KTILE), np.arange(KTILE), indexing="ij")
    trimask = np.where(kk <= qq, 0.0, MASK_NEG).astype(np.float32)

    # attn_outT row layout per pair-tile t: rows 0:64 head t, 64:128 head t+4
    sel = np.zeros((QH, FQ), np.float32)
    for t in range(4):
        sel[t, t * 128:t * 128 + 64] = 1.0
        sel[t + 4, t * 128 + 64:(t + 1) * 128] = 1.0

    import ml_dtypes
    bf16 = ml_dtypes.bfloat16
    cos2 = cos2.astype(bf16)
    sin2s = sin2s.astype(bf16)
    sel = sel.astype(bf16)
    xT = [np.ascontiguousarray(x[b].T.astype(bf16)) for b in range(B)]
    in_maps = []
    for c in range(NCORES):
        b, g = c // NG, c % NG
        # pair-tile column order: heads (t, t+4) interleaved per 128-col tile
        qcols = []
        wrows = []
        for t in range(4):
            for h in (8 * g + t, 8 * g + t + 4):
                qcols.append(wq[:, h * HD:(h + 1) * HD])
                wrows.append(wo[h * HD:(h + 1) * HD, :])
        wq_p = np.ascontiguousarray(np.concatenate(qcols, axis=1).astype(bf16))
        wo_p = np.ascontiguousarray(np.concatenate(wrows, axis=0).astype(bf16))
        kv0 = 2 * g
        wkv_p = np.ascontiguousarray(np.concatenate(
            [wk[:, kv0 * HD:(kv0 + 2) * HD], wv[:, kv0 * HD:(kv0 + 2) * HD]],
            axis=1).astype(bf16))
        in_maps.append({
            "xT": xT[b], "wq": wq_p, "wkv": wkv_p, "wo": wo_p,
            "cos2": cos2, "sin2s": sin2s, "trimask": trimask, "sel": sel,
        })
    return in_maps


def run(x, wq, wk, wv, wo, trace=False, **trace_kwargs):
    nc = _get_program()
    in_maps = _host_inputs(x, wq, wk, wv, wo)
    res = run_bass_kernel_spmd(nc, in_maps, list(range(NCORES)),
                               trace=trace, **trace_kwargs)
    out = np.empty((B, S, D), np.float32)
    for b in range(B):
        for r in range(NG):
            shard = res.results[b * NG + r]["out"]  # [8*64, D]
            for h in range(2 * NSLAB):
                out[b, h * 256 + r * 64:h * 256 + (r + 1) * 64, :] = \
                    shard[h * 64:(h + 1) * 64, :]
    return out, res


def kernel(x, wq, wk, wv, wo):
    out, _ = run(x, wq, wk, wv, wo)
    return out.astype(np.float32)



# revision 9
# speedup vs baseline: 1.3062x; 1.3062x over previous
"""Tensor-parallel GQA multi-head attention (RoPE + causal softmax) for 8 trn2 cores.

Sharding v2: every core handles BOTH batches with 4 q-heads / 1 kv-head:
core c owns q-heads {4c..4c+3} (kv-head c) of batches 0 and 1. Attention
runs in transposed (feature-major) layout with flash-style causal tiling.
Per 512-token slab, the 8 cores exchange their normalized attention outputs
with one AllToAll (bf16, 512KB) so that core c ends up with ALL 2048
attention features for its 128-position output stripe (batch c//4, stripe
c%4); it then applies the full wo to produce disjoint output rows. No
reduction collective is needed.
"""

import sys

sys.path.insert(0, "/opt/trn_rl_repo")

import numpy as np

import concourse.bass as bass
import concourse.bacc as bacc
import concourse.mybir as mybir
from concourse import tile
from concourse.bass_utils import run_bass_kernel_spmd

B, S, D = 2, 2048, 2048
N_HEADS, N_KV, HD = 32, 8, 64
NCORES = 8
QH = 4    # q-heads per core
FQ = QH * HD       # 256 q-feature cols per core
FKV = 2 * HD       # 128 (K then V) per core
SCALE = 1.0 / 8.0  # 1/sqrt(HD)

QTILE = 512
KTILE = 128
NSLAB = S // QTILE  # 4
ND = D // 128       # 16 contraction chunks

F32 = mybir.dt.float32
EXP = mybir.ActivationFunctionType.Exp
BF16 = mybir.dt.bfloat16
MMD = BF16
LE = mybir.AluOpType.is_ge


def _build_kernel(tc, io):
    nc = tc.nc
    xT, wq, wkv, wo = io["xT"], io["wq"], io["wkv"], io["wo"]
    cos2, sin2s, sel = io["cos2"], io["sin2s"], io["sel"]
    out_full = io["out"]
    single = bool(io.get("single"))

    # ---------------- pools ----------------
    const = tc.alloc_tile_pool(name="const", bufs=1)
    wpool = tc.alloc_tile_pool(name="wpool", bufs=1, side="right")
    kvp = tc.alloc_tile_pool(name="kvp", bufs=1)
    xpool = tc.alloc_tile_pool(name="xpool", bufs=2)
    qpool = tc.alloc_tile_pool(name="qpool", bufs=2)
    aop = tc.alloc_tile_pool(name="aop", bufs=2, side="right")
    rp = tc.alloc_tile_pool(name="rp", bufs=2)
    pexp = tc.alloc_tile_pool(name="pexp", bufs=3)
    evac = tc.alloc_tile_pool(name="evac", bufs=2)
    aogp = tc.alloc_tile_pool(name="aogp", bufs=2, side="right")
    dram = tc.alloc_tile_pool(name="dram", bufs=1, space="DRAM")

    psM = tc.alloc_tile_pool(name="psM", bufs=2, space="PSUM")
    psS = tc.alloc_tile_pool(name="psS", bufs=2, space="PSUM")
    psO = tc.alloc_tile_pool(name="psO", bufs=1, space="PSUM")

    # ---------------- constants ----------------
    sel_t = const.tile([QH, 2 * KTILE], MMD)
    nc.sync.dma_start(sel_t[:], sel[:])
    cos2_t = const.tile([128, S], MMD)
    nc.sync.dma_start(cos2_t[:], cos2[:])
    sin2s_t = const.tile([128, S], MMD)
    nc.sync.dma_start(sin2s_t[:], sin2s[:])
    ident = const.tile([128, 64], F32)
    nc.gpsimd.memset(ident[:], 0.0)
    for p in (0, 64):
        nc.gpsimd.affine_select(
            out=ident[p:p + 64, :], in_=ident[p:p + 64, :],
            compare_op=mybir.AluOpType.not_equal,
            fill=1.0, base=0, pattern=[[-1, 64]], channel_multiplier=1,
        )

    # small projection weights (wq 2 tiles x 16, wkv 1 x 16)
    WQ = {}
    for t in range(2):
        for k in range(ND):
            w = wpool.tile([128, 128], MMD, name=f"wq{t}_{k}")
            nc.sync.dma_start(w[:], wq[k * 128:(k + 1) * 128,
                                        t * 128:(t + 1) * 128])
            WQ[t, k] = w
    WKV = {}
    for k in range(ND):
        w = wpool.tile([128, 128], MMD, name=f"wkv{k}")
        nc.sync.dma_start(w[:], wkv[k * 128:(k + 1) * 128, :])
        WKV[k] = w

    # full wo (loaded via the scalar DMA queue; scalar is idle early on)
    WO = {}
    for fc in range(ND):
        for dn in range(4):
            w = wpool.tile([128, QTILE], MMD, name=f"wo{fc}_{dn}")
            nc.scalar.dma_start(
                w[:], wo[fc * 128:(fc + 1) * 128,
                         dn * QTILE:(dn + 1) * QTILE])
            WO[fc, dn] = w

    # persistent K/V cache tiles
    KK = [kvp.tile([128, S], MMD, name=f"kk{b}") for b in range(B)]
    VA = {}
    for b in range(B):
        for i in range(S // KTILE):
            VA[b, i] = kvp.tile([128, HD + 1], MMD, name=f"va{b}_{i}")

    # A2A dram tiles (one pair per slab)
    a2a_in = [dram.tile([FQ * NCORES, KTILE], MMD, name=f"ain{j}")
              for j in range(NSLAB)]
    a2a_out = [dram.tile([FQ * NCORES, KTILE], MMD, name=f"aout{j}")
               for j in range(NSLAB)]

    AO = {}   # per (b, t) slab-local attention output, feature-major
    QT = {}

    def rope(dst, rows, qs, tab_qs):
        # dst[rows, qs] = dst*cos + swap32(dst)*sin  (feature-major RoPE);
        # qs indexes dst columns, tab_qs the (global-position) rope tables
        n = rows[1] - rows[0]
        qsw = rp.tile([128, QTILE], MMD, name="qsw", tag="qsw")
        for p in range(rows[0], rows[1], 64):
            q0 = p - rows[0]
            nc.sync.dma_start(qsw[q0:q0 + 32, :], dst[p + 32:p + 64, qs])
            nc.sync.dma_start(qsw[q0 + 32:q0 + 64, :], dst[p:p + 32, qs])
        t1 = rp.tile([128, QTILE], F32, name="t1", tag="t1")
        nc.vector.tensor_mul(t1[:n], dst[rows[0]:rows[1], qs],
                             cos2_t[rows[0]:rows[1], tab_qs])
        t2 = rp.tile([128, QTILE], F32, name="t2", tag="t2")
        nc.vector.tensor_mul(t2[:n], qsw[:n], sin2s_t[rows[0]:rows[1], tab_qs])
        nc.vector.tensor_add(dst[rows[0]:rows[1], qs], t1[:n], t2[:n])

    def proj(b, j):
        qs = slice(j * QTILE, (j + 1) * QTILE)
        xts = []
        for k in range(ND):
            xt = xpool.tile([128, QTILE], MMD, name="xt", tag=f"xt{k}")
            nc.sync.dma_start(
                xt[:], xT[b * D + k * 128:b * D + (k + 1) * 128, qs])
            xts.append(xt)
        for t in range(2):
            ps = psM.tile([128, QTILE], F32, name="psq", tag="mm")
            for k in range(ND):
                nc.tensor.matmul(ps[:], WQ[t, k][:], xts[k][:],
                                 start=(k == 0), stop=(k == ND - 1))
            qt = qpool.tile([128, QTILE], MMD, name="qt", tag=f"qt{b}_{t}")
            QT[b, t] = qt
            nc.vector.tensor_copy(qt[:], ps[:])
            rope(qt, (0, 128), slice(0, QTILE), qs)
        ps = psM.tile([128, QTILE], F32, name="pskv", tag="mm")
        for k in range(ND):
            nc.tensor.matmul(ps[:], WKV[k][:], xts[k][:],
                             start=(k == 0), stop=(k == ND - 1))
        nc.vector.tensor_copy(KK[b][0:64, qs], ps[0:64, :])
        rope(KK[b], (0, 64), qs, qs)
        # duplicate roped K into rows 64:128 (for the row-tiled score pair)
        nc.sync.dma_start(KK[b][64:128, qs], KK[b][0:64, qs])
        vv = rp.tile([128, QTILE], F32, name="vv", tag="vv")
        nc.vector.tensor_copy(vv[64:128, :], ps[64:128, :])
        for c in range(4):
            i = 4 * j + c
            tp = psM.tile([128, QTILE], F32, name="tp", tag="mm")
            nc.tensor.matmul(tp[:, 0:HD], vv[64:128, c * 128:(c + 1) * 128],
                             ident[64:128, :], is_transpose=True,
                             start=True, stop=True)
            va = VA[b, i]
            nc.vector.tensor_copy(va[:, 0:HD], tp[:, 0:HD])
            nc.vector.memset(va[:, HD:HD + 1], 1.0)

    def attn(b, j):
        nkt = 4 * j + 4
        for t in range(2):
            oA = psO.tile([HD + 1, QTILE], F32, name="oA", tag="oA")
            oB = psO.tile([HD + 1, QTILE], F32, name="oB", tag="oB")
            sabs = {}

            def scores(i):
                r = i - 4 * j
                off = max(r, 0) * KTILE
                ks = slice(i * KTILE, (i + 1) * KTILE)
                sAB = psS.tile([128, 2 * QTILE], F32, name="sAB", tag="sAB")
                nc.tensor.matmul(sAB[:, off:QTILE], KK[b][0:64, ks],
                                 QT[b, t][0:64, off:], start=True, stop=True,
                                 tile_position=(0, 0))
                nc.tensor.matmul(sAB[:, QTILE + off:], KK[b][64:128, ks],
                                 QT[b, t][64:128, off:], start=True, stop=True,
                                 tile_position=(64, 0))
                sabs[i] = sAB

            scores(0)
            for i in range(nkt):
                r = i - 4 * j
                off = max(r, 0) * KTILE
                if i + 1 < nkt:
                    scores(i + 1)
                sAB = sabs.pop(i)
                pAB = pexp.tile([128, 2 * QTILE], MMD, name="pAB", tag="pAB")
                nc.scalar.activation(pAB[:, off:], sAB[:, off:], EXP,
                                     scale=SCALE)
                if r >= 0:
                    for h in (off, QTILE + off):
                        # keep where q - k >= 0 (causal), zero-fill above diag
                        nc.gpsimd.affine_select(
                            out=pAB[:, h:h + KTILE], in_=pAB[:, h:h + KTILE],
                            compare_op=LE, fill=0.0, base=0,
                            pattern=[[1, KTILE]], channel_multiplier=-1)
                nc.tensor.matmul(oA[:, off:], VA[b, i][:], pAB[:, off:QTILE],
                                 start=(i == 0), stop=(i == nkt - 1))
                nc.tensor.matmul(oB[:, off:], VA[b, i][:], pAB[:, QTILE + off:],
                                 start=(i == 0), stop=(i == nkt - 1))
            tA = evac.tile([HD + 1, QTILE], MMD, name="tA", tag="tA")
            tB = evac.tile([HD + 1, QTILE], MMD, name="tB", tag="tB")
            nc.vector.tensor_copy(tA[:], oA[:])
            nc.vector.tensor_copy(tB[:], oB[:])
            ao = AO[b, t]
            nc.scalar.dma_start(ao[0:64, :], tA[0:64, :])
            nc.scalar.dma_start(ao[64:128, :], tB[0:64, :])
            dn = AO[b, "dn"]
            nc.scalar.dma_start(dn[t:t + 1, :], tA[64:65, :])
            nc.scalar.dma_start(dn[2 + t:3 + t, :], tB[64:65, :])

    def finish(j):
        # normalize, build the A2A input, kick the A2A
        for b in range(B):
            dn = AO[b, "dn"]
            dnR = evac.tile([QH, QTILE], F32, name="dnR", tag="dnR")
            nc.vector.reciprocal(dnR[:], dn[:])
            dnRb = evac.tile([QH, QTILE], MMD, name="dnRb", tag="dnRb")
            nc.vector.tensor_copy(dnRb[:], dnR[:])
            for t in range(2):
                bc = psM.tile([128, QTILE], F32, name="bc", tag="mm")
                nc.tensor.matmul(bc[:], sel_t[:, t * 128:(t + 1) * 128],
                                 dnRb[:], start=True, stop=True)
                nc.vector.tensor_mul(AO[b, t][:], AO[b, t][:], bc[:])
        for d in range(NCORES):
            bd, g = d // 4, d % 4
            for t in range(2):
                nc.sync.dma_start(
                    a2a_in[j][FQ * d + 128 * t:FQ * d + 128 * (t + 1), :],
                    AO[bd, t][:, g * KTILE:(g + 1) * KTILE])
        if single:
            nc.sync.dma_start(a2a_out[j][:], a2a_in[j][:])
        else:
            nc.gpsimd.collective_compute(
                "AllToAll", mybir.AluOpType.bypass,
                replica_groups=[list(range(NCORES))],
                ins=[a2a_in[j][:]], outs=[a2a_out[j][:]],
            )

    def wo_slab(j):
        aogs = []
        for fc in range(ND):
            aog = aogp.tile([128, KTILE], MMD, name="aog", tag=f"aog{fc}")
            nc.sync.dma_start(aog[:], a2a_out[j][fc * 128:(fc + 1) * 128, :])
            aogs.append(aog)
        for dn in range(4):
            psW = psM.tile([128, QTILE], F32, name="psW", tag="mm")
            for fc in range(ND):
                nc.tensor.matmul(psW[:], aogs[fc][:], WO[fc, dn][:],
                                 start=(fc == 0), stop=(fc == ND - 1))
            og = evac.tile([128, QTILE], F32, name="og", tag="og")
            nc.vector.tensor_copy(og[:], psW[:])
            nc.gpsimd.dma_start(
                out_full[j * 128:(j + 1) * 128, dn * QTILE:(dn + 1) * QTILE],
                og[:])

    for j in range(NSLAB):
        for b in range(B):
            AO[b, 0] = aop.tile([128, QTILE], MMD, name=f"ao{b}0",
                                tag=f"ao{b}0")
            AO[b, 1] = aop.tile([128, QTILE], MMD, name=f"ao{b}1",
                                tag=f"ao{b}1")
            AO[b, "dn"] = aop.tile([QH, QTILE], MMD, name=f"dn{b}",
                                   tag=f"dn{b}")
            proj(b, j)
            attn(b, j)
        finish(j)
        if j > 0:
            wo_slab(j - 1)
    wo_slab(NSLAB - 1)

    for p in (psO, psS, psM, dram, aogp, evac, pexp, rp, aop, qpool, xpool,
              kvp, wpool, const):
        p.release()


def _build(single=False):
    nc = bacc.Bacc("TRN2", target_bir_lowering=False, debug=False,
                   num_devices=1 if single else NCORES)
    io = {
        "xT": nc.dram_tensor("xT", [B * D, S], BF16, kind="ExternalInput").ap(),
        "wq": nc.dram_tensor("wq", [D, FQ], BF16, kind="ExternalInput").ap(),
        "wkv": nc.dram_tensor("wkv", [D, FKV], BF16, kind="ExternalInput").ap(),
        "wo": nc.dram_tensor("wo", [D, D], BF16, kind="ExternalInput").ap(),
        "cos2": nc.dram_tensor("cos2", [128, S], BF16, kind="ExternalInput").ap(),
        "sin2s": nc.dram_tensor("sin2s", [128, S], BF16, kind="ExternalInput").ap(),
        "sel": nc.dram_tensor("sel", [QH, 2 * KTILE], BF16,
                              kind="ExternalInput").ap(),
        "out": nc.dram_tensor("out", [NSLAB * 128, D], F32,
                              kind="ExternalOutput").ap(),
    }
    io["single"] = single
    with tile.TileContext(nc) as tc:
        _build_kernel(tc, io)
    nc.compile()
    return nc


_CACHE = {}


def _get_program():
    if "nc" not in _CACHE:
        _CACHE["nc"] = _build()
    return _CACHE["nc"]


def _host_inputs(x, wq, wk, wv, wo):
    x = np.ascontiguousarray(x, np.float32)
    inv = 1.0 / (10000.0 ** (np.arange(0, HD, 2, dtype=np.float64) / HD))
    pos = np.arange(S, dtype=np.float64)
    freqs = np.outer(pos, inv)                      # [S, 32]
    emb = np.concatenate([freqs, freqs], axis=1)    # [S, 64]
    cos = np.cos(emb).T.astype(np.float32)          # [64, S]
    sin = np.sin(emb).T.astype(np.float32)
    cos2 = np.concatenate([cos, cos], axis=0)       # [128, S]
    sin2s = np.concatenate([-sin[:32], sin[32:], -sin[:32], sin[32:]], axis=0)

    # denominator broadcast selector: AO[t] rows 0:64 <- dn row t,
    # rows 64:128 <- dn row 2+t
    sel = np.zeros((QH, 2 * KTILE), np.float32)
    for t in range(2):
        sel[t, t * 128:t * 128 + 64] = 1.0
        sel[2 + t, t * 128 + 64:(t + 1) * 128] = 1.0

    import ml_dtypes
    bf16 = ml_dtypes.bfloat16
    cos2 = cos2.astype(bf16)
    sin2s = sin2s.astype(bf16)
    sel = sel.astype(bf16)
    xT = np.ascontiguousarray(
        np.concatenate([x[0].T, x[1].T], axis=0).astype(bf16))  # [2D, S]

    # wo rows ordered to match the gathered A2A feature order:
    # src core cc contributes heads (4cc+t, 4cc+t+2) for t in (0, 1)
    wrows = []
    for cc in range(NCORES):
        for t in range(2):
            for h in (4 * cc + t, 4 * cc + t + 2):
                wrows.append(wo[h * HD:(h + 1) * HD, :])
    wo_p = np.ascontiguousarray(np.concatenate(wrows, axis=0).astype(bf16))

    in_maps = []
    for c in range(NCORES):
        qcols = []
        for t in range(2):
            for h in (4 * c + t, 4 * c + t + 2):
                qcols.append(wq[:, h * HD:(h + 1) * HD])
        wq_p = np.ascontiguousarray(np.concatenate(qcols, axis=1).astype(bf16))
        wkv_p = np.ascontiguousarray(np.concatenate(
            [wk[:, c * HD:(c + 1) * HD], wv[:, c * HD:(c + 1) * HD]],
            axis=1).astype(bf16))
        in_maps.append({
            "xT": xT, "wq": wq_p, "wkv": wkv_p, "wo": wo_p,
            "cos2": cos2, "sin2s": sin2s, "sel": sel,
        })
    return in_maps


def run(x, wq, wk, wv, wo, trace=False, **trace_kwargs):
    nc = _get_program()
    in_maps = _host_inputs(x, wq, wk, wv, wo)
    res = run_bass_kernel_spmd(nc, in_maps, list(range(NCORES)),
                               trace=trace, **trace_kwargs)
    out = np.empty((B, S, D), np.float32)
    for c in range(NCORES):
        bo, g = c // 4, c % 4
        shard = res.results[c]["out"]  # [512, D]
        for j in range(NSLAB):
            out[bo, j * QTILE + g * 128:j * QTILE + (g + 1) * 128, :] = \
                shard[j * 128:(j + 1) * 128, :]
    return out, res


def kernel(x, wq, wk, wv, wo):
    out, _ = run(x, wq, wk, wv, wo)
    return out.astype(np.float32)


# revision 17
# speedup vs baseline: 1.3826x; 1.0585x over previous
"""Tensor-parallel GQA multi-head attention (RoPE + causal softmax) for 8 trn2 cores.

Sharding v2: every core handles BOTH batches with 4 q-heads / 1 kv-head:
core c owns q-heads {4c..4c+3} (kv-head c) of batches 0 and 1. Attention
runs in transposed (feature-major) layout with flash-style causal tiling.
Per 512-token slab, the 8 cores exchange their normalized attention outputs
with one AllToAll (bf16, 512KB) so that core c ends up with ALL 2048
attention features for its 128-position output stripe (batch c//4, stripe
c%4); it then applies the full wo to produce disjoint output rows. No
reduction collective is needed.
"""

import sys

sys.path.insert(0, "/opt/trn_rl_repo")

import numpy as np

import concourse.bass as bass
import concourse.bacc as bacc
import concourse.mybir as mybir
from concourse import tile
from concourse.bass_utils import run_bass_kernel_spmd

B, S, D = 2, 2048, 2048
N_HEADS, N_KV, HD = 32, 8, 64
NCORES = 8
QH = 4    # q-heads per core
FQ = QH * HD       # 256 q-feature cols per core
FKV = 2 * HD       # 128 (K then V) per core
SCALE = 1.0 / 8.0  # 1/sqrt(HD)

QTILE = 512
KTILE = 128
NSLAB = S // QTILE  # 4
ND = D // 128       # 16 contraction chunks

F32 = mybir.dt.float32
EXP = mybir.ActivationFunctionType.Exp
BF16 = mybir.dt.bfloat16
MMD = BF16
LE = mybir.AluOpType.is_ge


def _build_kernel(tc, io):
    nc = tc.nc
    xT, wq, wkv, wo = io["xT"], io["wq"], io["wkv"], io["wo"]
    cos2, sin2s, sel = io["cos2"], io["sin2s"], io["sel"]
    out_full = io["out"]
    single = bool(io.get("single"))

    # ---------------- pools ----------------
    const = tc.alloc_tile_pool(name="const", bufs=1)
    wpool = tc.alloc_tile_pool(name="wpool", bufs=1, side="right")
    kvp = tc.alloc_tile_pool(name="kvp", bufs=1)
    xpool = tc.alloc_tile_pool(name="xpool", bufs=2)
    qpool = tc.alloc_tile_pool(name="qpool", bufs=2)
    aop = tc.alloc_tile_pool(name="aop", bufs=2, side="right")
    rp = tc.alloc_tile_pool(name="rp", bufs=2)
    pexp = tc.alloc_tile_pool(name="pexp", bufs=3)
    evac = tc.alloc_tile_pool(name="evac", bufs=2)
    aogp = tc.alloc_tile_pool(name="aogp", bufs=2, side="right")
    dram = tc.alloc_tile_pool(name="dram", bufs=1, space="DRAM")

    psM = tc.alloc_tile_pool(name="psM", bufs=2, space="PSUM")
    psS = tc.alloc_tile_pool(name="psS", bufs=2, space="PSUM")
    psO = tc.alloc_tile_pool(name="psO", bufs=1, space="PSUM")

    # ------- constants + weights; DMA order tuned for fast start -------
    cos2_t = const.tile([128, S], MMD)
    nc.sync.dma_start(cos2_t[:], cos2[:])
    sin2s_t = const.tile([128, S], MMD)
    nc.sync.dma_start(sin2s_t[:], sin2s[:])
    ident = const.tile([128, 64], F32)
    nc.gpsimd.memset(ident[:], 0.0)
    for p in (0, 64):
        nc.gpsimd.affine_select(
            out=ident[p:p + 64, :], in_=ident[p:p + 64, :],
            compare_op=mybir.AluOpType.not_equal,
            fill=1.0, base=0, pattern=[[-1, 64]], channel_multiplier=1,
        )
    # causal keep-mask for the diagonal blocks (1 where q >= k)
    trimask_t = const.tile([KTILE, KTILE], MMD)
    nc.gpsimd.memset(trimask_t[:], 1.0)
    nc.gpsimd.affine_select(
        out=trimask_t[:], in_=trimask_t[:],
        compare_op=mybir.AluOpType.is_ge,
        fill=0.0, base=0, pattern=[[1, KTILE]], channel_multiplier=-1,
    )

    # x slab for (b=0, j=0) interleaved with the projection weights so the
    # first matmul group can start within a few microseconds
    xts00 = []
    WQ = {}
    WKV = {}
    for k in range(ND):
        xt = xpool.tile([128, QTILE], MMD, name="xt", tag=f"xt{k}")
        nc.sync.dma_start(xt[:], xT[k * 128:(k + 1) * 128, 0:QTILE])
        xts00.append(xt)
        for t in range(2):
            w = wpool.tile([128, 128], MMD, name=f"wq{t}_{k}")
            nc.sync.dma_start(w[:], wq[k * 128:(k + 1) * 128,
                                        t * 128:(t + 1) * 128])
            WQ[t, k] = w
        w = wpool.tile([128, 128], MMD, name=f"wkv{k}")
        nc.sync.dma_start(w[:], wkv[k * 128:(k + 1) * 128, :])
        WKV[k] = w

    sel_t = const.tile([2 * QH, 4 * KTILE], MMD)
    nc.sync.dma_start(sel_t[:], sel[:])

    # full wo (loaded via the scalar DMA queue; scalar is idle early on)
    WO = {}
    for fc in range(ND):
        for dn in range(4):
            w = wpool.tile([128, QTILE], MMD, name=f"wo{fc}_{dn}")
            nc.scalar.dma_start(
                w[:], wo[fc * 128:(fc + 1) * 128,
                         dn * QTILE:(dn + 1) * QTILE])
            WO[fc, dn] = w

    # persistent K/V cache tiles
    KK = [kvp.tile([128, S], MMD, name=f"kk{b}") for b in range(B)]
    VA = {}
    for b in range(B):
        for i in range(S // KTILE):
            VA[b, i] = kvp.tile([128, HD + 1], MMD, name=f"va{b}_{i}")

    # A2A dram tiles (one pair per slab)
    a2a_in = [dram.tile([FQ * NCORES, KTILE], MMD, name=f"ain{j}")
              for j in range(NSLAB)]
    a2a_out = [dram.tile([FQ * NCORES, KTILE], MMD, name=f"aout{j}")
               for j in range(NSLAB)]

    AO = {}   # per (b, t) slab-local attention output, feature-major
    QT = {}

    def rope(dst, rows, qs, tab_qs):
        # dst[rows, qs] = dst*cos + swap32(dst)*sin  (feature-major RoPE);
        # qs indexes dst columns, tab_qs the (global-position) rope tables
        n = rows[1] - rows[0]
        qsw = rp.tile([128, QTILE], MMD, name="qsw", tag="qsw")
        for p in range(rows[0], rows[1], 64):
            q0 = p - rows[0]
            nc.sync.dma_start(qsw[q0:q0 + 32, :], dst[p + 32:p + 64, qs])
            nc.sync.dma_start(qsw[q0 + 32:q0 + 64, :], dst[p:p + 32, qs])
        t1 = rp.tile([128, QTILE], F32, name="t1", tag="t1")
        nc.vector.tensor_mul(t1[:n], dst[rows[0]:rows[1], qs],
                             cos2_t[rows[0]:rows[1], tab_qs])
        t2 = rp.tile([128, QTILE], F32, name="t2", tag="t2")
        nc.vector.tensor_mul(t2[:n], qsw[:n], sin2s_t[rows[0]:rows[1], tab_qs])
        nc.vector.tensor_add(dst[rows[0]:rows[1], qs], t1[:n], t2[:n])

    def proj(b, j, xts=None):
        qs = slice(j * QTILE, (j + 1) * QTILE)
        if xts is None:
            xts = []
            for k in range(ND):
                xt = xpool.tile([128, QTILE], MMD, name="xt", tag=f"xt{k}")
                nc.sync.dma_start(
                    xt[:], xT[b * D + k * 128:b * D + (k + 1) * 128, qs])
                xts.append(xt)
        for t in range(2):
            ps = psM.tile([128, QTILE], F32, name="psq", tag="mm")
            for k in range(ND):
                nc.tensor.matmul(ps[:], WQ[t, k][:], xts[k][:],
                                 start=(k == 0), stop=(k == ND - 1))
            qt = qpool.tile([128, QTILE], MMD, name="qt", tag=f"qt{b}_{t}")
            QT[b, t] = qt
            nc.vector.tensor_copy(qt[:], ps[:])
            rope(qt, (0, 128), slice(0, QTILE), qs)
        ps = psM.tile([128, QTILE], F32, name="pskv", tag="mm")
        for k in range(ND):
            nc.tensor.matmul(ps[:], WKV[k][:], xts[k][:],
                             start=(k == 0), stop=(k == ND - 1))
        nc.vector.tensor_copy(KK[b][0:64, qs], ps[0:64, :])
        rope(KK[b], (0, 64), qs, qs)
        # duplicate roped K into rows 64:128 (for the row-tiled score pair)
        nc.sync.dma_start(KK[b][64:128, qs], KK[b][0:64, qs])
        vv = rp.tile([128, QTILE], F32, name="vv", tag="vv")
        nc.vector.tensor_copy(vv[64:128, :], ps[64:128, :])
        for c in range(4):
            i = 4 * j + c
            tp = psM.tile([128, QTILE], F32, name="tp", tag="mm")
            nc.tensor.matmul(tp[:, 0:HD], vv[64:128, c * 128:(c + 1) * 128],
                             ident[64:128, :], is_transpose=True,
                             start=True, stop=True)
            va = VA[b, i]
            nc.vector.tensor_copy(va[:, 0:HD], tp[:, 0:HD])
            nc.vector.memset(va[:, HD:HD + 1], 1.0)

    def attn(b, j):
        nkt = 4 * j + 4
        for t in range(2):
            oA = psO.tile([HD + 1, QTILE], F32, name="oA", tag="oA")
            oB = psO.tile([HD + 1, QTILE], F32, name="oB", tag="oB")
            sabs = {}

            def scores(i):
                r = i - 4 * j
                off = max(r, 0) * KTILE
                ks = slice(i * KTILE, (i + 1) * KTILE)
                sAB = psS.tile([128, 2 * QTILE], F32, name="sAB", tag="sAB")
                nc.tensor.matmul(sAB[:, off:QTILE], KK[b][0:64, ks],
                                 QT[b, t][0:64, off:], start=True, stop=True,
                                 tile_position=(0, 0))
                nc.tensor.matmul(sAB[:, QTILE + off:], KK[b][64:128, ks],
                                 QT[b, t][64:128, off:], start=True, stop=True,
                                 tile_position=(64, 0))
                sabs[i] = sAB

            scores(0)
            for i in range(nkt):
                r = i - 4 * j
                off = max(r, 0) * KTILE
                if i + 1 < nkt:
                    scores(i + 1)
                sAB = sabs.pop(i)
                pAB = pexp.tile([128, 2 * QTILE], MMD, name="pAB", tag="pAB")
                nc.scalar.activation(pAB[:, off:], sAB[:, off:], EXP,
                                     scale=SCALE)
                if r >= 0:
                    for h in (off, QTILE + off):
                        # zero the strictly-upper triangle (causal mask)
                        nc.vector.tensor_mul(pAB[:, h:h + KTILE],
                                             pAB[:, h:h + KTILE],
                                             trimask_t[:])
                nc.tensor.matmul(oA[:, off:], VA[b, i][:], pAB[:, off:QTILE],
                                 start=(i == 0), stop=(i == nkt - 1))
                nc.tensor.matmul(oB[:, off:], VA[b, i][:], pAB[:, QTILE + off:],
                                 start=(i == 0), stop=(i == nkt - 1))
            tA = evac.tile([HD + 1, QTILE], MMD, name="tA", tag="tA")
            tB = evac.tile([HD + 1, QTILE], MMD, name="tB", tag="tB")
            nc.vector.tensor_copy(tA[:], oA[:])
            nc.vector.tensor_copy(tB[:], oB[:])
            ao = AO[b, t]
            nc.sync.dma_start(ao[0:64, :], tA[0:64, :])
            nc.sync.dma_start(ao[64:128, :], tB[0:64, :])
            dn = AO["dn"]
            nc.sync.dma_start(dn[4 * b + t:4 * b + t + 1, :], tA[64:65, :])
            nc.sync.dma_start(dn[4 * b + 2 + t:4 * b + 3 + t, :], tB[64:65, :])

    def finish(j):
        # normalize, build the A2A input, kick the A2A
        dn = AO["dn"]
        dnR = evac.tile([2 * QH, QTILE], F32, name="dnR", tag="dnR")
        nc.vector.reciprocal(dnR[:], dn[:])
        dnRb = evac.tile([2 * QH, QTILE], MMD, name="dnRb", tag="dnRb")
        nc.vector.tensor_copy(dnRb[:], dnR[:])
        for b in range(B):
            for t in range(2):
                bc = psM.tile([128, QTILE], F32, name="bc", tag="mm")
                nc.tensor.matmul(
                    bc[:], sel_t[:, (2 * b + t) * 128:(2 * b + t + 1) * 128],
                    dnRb[:], start=True, stop=True)
                nc.vector.tensor_mul(AO[b, t][:], AO[b, t][:], bc[:])
        for d in range(NCORES):
            bd, g = d // 4, d % 4
            for t in range(2):
                nc.gpsimd.dma_start(
                    a2a_in[j][FQ * d + 128 * t:FQ * d + 128 * (t + 1), :],
                    AO[bd, t][:, g * KTILE:(g + 1) * KTILE])
        if single:
            nc.sync.dma_start(a2a_out[j][:], a2a_in[j][:])
        else:
            nc.gpsimd.collective_compute(
                "AllToAll", mybir.AluOpType.bypass,
                replica_groups=[list(range(NCORES))],
                ins=[a2a_in[j][:]], outs=[a2a_out[j][:]],
            )

    def wo_slab(j):
        aogs = []
        for fc in range(ND):
            aog = aogp.tile([128, KTILE], MMD, name="aog", tag=f"aog{fc}")
            nc.sync.dma_start(aog[:], a2a_out[j][fc * 128:(fc + 1) * 128, :])
            aogs.append(aog)
        for dn in range(4):
            psW = psM.tile([128, QTILE], F32, name="psW", tag="mm")
            for fc in range(ND):
                nc.tensor.matmul(psW[:], aogs[fc][:], WO[fc, dn][:],
                                 start=(fc == 0), stop=(fc == ND - 1))
            og = evac.tile([128, QTILE], F32, name="og", tag="og")
            nc.vector.tensor_copy(og[:], psW[:])
            nc.gpsimd.dma_start(
                out_full[j * 128:(j + 1) * 128, dn * QTILE:(dn + 1) * QTILE],
                og[:])

    for j in range(NSLAB):
        AO["dn"] = aop.tile([2 * QH, QTILE], MMD, name="dn", tag="dn")
        for b in range(B):
            AO[b, 0] = aop.tile([128, QTILE], MMD, name=f"ao{b}0",
                                tag=f"ao{b}0")
            AO[b, 1] = aop.tile([128, QTILE], MMD, name=f"ao{b}1",
                                tag=f"ao{b}1")
            proj(b, j, xts00 if (b == 0 and j == 0) else None)
            attn(b, j)
        finish(j)
        if j > 0:
            wo_slab(j - 1)
    wo_slab(NSLAB - 1)

    for p in (psO, psS, psM, dram, aogp, evac, pexp, rp, aop, qpool, xpool,
              kvp, wpool, const):
        p.release()


def _build(single=False):
    nc = bacc.Bacc("TRN2", target_bir_lowering=False, debug=False,
                   num_devices=1 if single else NCORES)
    io = {
        "xT": nc.dram_tensor("xT", [B * D, S], BF16, kind="ExternalInput").ap(),
        "wq": nc.dram_tensor("wq", [D, FQ], BF16, kind="ExternalInput").ap(),
        "wkv": nc.dram_tensor("wkv", [D, FKV], BF16, kind="ExternalInput").ap(),
        "wo": nc.dram_tensor("wo", [D, D], BF16, kind="ExternalInput").ap(),
        "cos2": nc.dram_tensor("cos2", [128, S], BF16, kind="ExternalInput").ap(),
        "sin2s": nc.dram_tensor("sin2s", [128, S], BF16, kind="ExternalInput").ap(),
        "sel": nc.dram_tensor("sel", [2 * QH, 4 * KTILE], BF16,
                              kind="ExternalInput").ap(),
        "out": nc.dram_tensor("out", [NSLAB * 128, D], F32,
                              kind="ExternalOutput").ap(),
    }
    io["single"] = single
    with tile.TileContext(nc) as tc:
        _build_kernel(tc, io)
    nc.compile()
    return nc


_CACHE = {}


def _get_program():
    if "nc" not in _CACHE:
        _CACHE["nc"] = _build()
    return _CACHE["nc"]


def _host_inputs(x, wq, wk, wv, wo):
    x = np.ascontiguousarray(x, np.float32)
    inv = 1.0 / (10000.0 ** (np.arange(0, HD, 2, dtype=np.float64) / HD))
    pos = np.arange(S, dtype=np.float64)
    freqs = np.outer(pos, inv)                      # [S, 32]
    emb = np.concatenate([freqs, freqs], axis=1)    # [S, 64]
    cos = np.cos(emb).T.astype(np.float32)          # [64, S]
    sin = np.sin(emb).T.astype(np.float32)
    cos2 = np.concatenate([cos, cos], axis=0)       # [128, S]
    sin2s = np.concatenate([-sin[:32], sin[32:], -sin[:32], sin[32:]], axis=0)

    # denominator broadcast selector: for (b, t) block, AO[b,t] rows 0:64
    # <- dn row 4b+t, rows 64:128 <- dn row 4b+2+t
    sel = np.zeros((2 * QH, 4 * KTILE), np.float32)
    for b in range(2):
        for t in range(2):
            blk = (2 * b + t) * 128
            sel[4 * b + t, blk:blk + 64] = 1.0
            sel[4 * b + 2 + t, blk + 64:blk + 128] = 1.0

    import ml_dtypes
    bf16 = ml_dtypes.bfloat16
    cos2 = cos2.astype(bf16)
    sin2s = sin2s.astype(bf16)
    sel = sel.astype(bf16)
    xT = np.ascontiguousarray(
        np.concatenate([x[0].T, x[1].T], axis=0).astype(bf16))  # [2D, S]

    # wo rows ordered to match the gathered A2A feature order:
    # src core cc contributes heads (4cc+t, 4cc+t+2) for t in (0, 1)
    wrows = []
    for cc in range(NCORES):
        for t in range(2):
            for h in (4 * cc + t, 4 * cc + t + 2):
                wrows.append(wo[h * HD:(h + 1) * HD, :])
    wo_p = np.ascontiguousarray(np.concatenate(wrows, axis=0).astype(bf16))

    in_maps = []
    for c in range(NCORES):
        qcols = []
        for t in range(2):
            for h in (4 * c + t, 4 * c + t + 2):
                qcols.append(wq[:, h * HD:(h + 1) * HD])
        wq_p = np.ascontiguousarray(np.concatenate(qcols, axis=1).astype(bf16))
        wkv_p = np.ascontiguousarray(np.concatenate(
            [wk[:, c * HD:(c + 1) * HD], wv[:, c * HD:(c + 1) * HD]],
            axis=1).astype(bf16))
        in_maps.append({
            "xT": xT, "wq": wq_p, "wkv": wkv_p, "wo": wo_p,
            "cos2": cos2, "sin2s": sin2s, "sel": sel,
        })
    return in_maps


def run(x, wq, wk, wv, wo, trace=False, **trace_kwargs):
    nc = _get_program()
    in_maps = _host_inputs(x, wq, wk, wv, wo)
    res = run_bass_kernel_spmd(nc, in_maps, list(range(NCORES)),
                               trace=trace, **trace_kwargs)
    out = np.empty((B, S, D), np.float32)
    for c in range(NCORES):
        bo, g = c // 4, c % 4
        shard = res.results[c]["out"]  # [512, D]
        for j in range(NSLAB):
            out[bo, j * QTILE + g * 128:j * QTILE + (g + 1) * 128, :] = \
                shard[j * 128:(j + 1) * 128, :]
    return out, res


def kernel(x, wq, wk, wv, wo):
    out, _ = run(x, wq, wk, wv, wo)
    return out.astype(np.float32)


# revision 21
# speedup vs baseline: 1.3829x; 1.0002x over previous
"""Tensor-parallel GQA multi-head attention (RoPE + causal softmax) for 8 trn2 cores.

Sharding v2: every core handles BOTH batches with 4 q-heads / 1 kv-head:
core c owns q-heads {4c..4c+3} (kv-head c) of batches 0 and 1. Attention
runs in transposed (feature-major) layout with flash-style causal tiling.
Per 512-token slab, the 8 cores exchange their normalized attention outputs
with one AllToAll (bf16, 512KB) so that core c ends up with ALL 2048
attention features for its 128-position output stripe (batch c//4, stripe
c%4); it then applies the full wo to produce disjoint output rows. No
reduction collective is needed.
"""

import sys

sys.path.insert(0, "/opt/trn_rl_repo")

import numpy as np

import concourse.bass as bass
import concourse.bacc as bacc
import concourse.mybir as mybir
from concourse import tile
from concourse.bass_utils import run_bass_kernel_spmd

B, S, D = 2, 2048, 2048
N_HEADS, N_KV, HD = 32, 8, 64
NCORES = 8
QH = 4    # q-heads per core
FQ = QH * HD       # 256 q-feature cols per core
FKV = 2 * HD       # 128 (K then V) per core
SCALE = 1.0 / 8.0  # 1/sqrt(HD)

QTILE = 512
KTILE = 128
NSLAB = S // QTILE  # 4
ND = D // 128       # 16 contraction chunks

F32 = mybir.dt.float32
EXP = mybir.ActivationFunctionType.Exp
BF16 = mybir.dt.bfloat16
MMD = BF16
LE = mybir.AluOpType.is_ge


def _build_kernel(tc, io):
    nc = tc.nc
    xT, wq, wkv, wo = io["xT"], io["wq"], io["wkv"], io["wo"]
    cos2, sin2s, sel = io["cos2"], io["sin2s"], io["sel"]
    out_full = io["out"]
    single = bool(io.get("single"))

    # ---------------- pools ----------------
    const = tc.alloc_tile_pool(name="const", bufs=1)
    wpool = tc.alloc_tile_pool(name="wpool", bufs=1, side="right")
    kvp = tc.alloc_tile_pool(name="kvp", bufs=1)
    xpool = tc.alloc_tile_pool(name="xpool", bufs=2)
    qpool = tc.alloc_tile_pool(name="qpool", bufs=2)
    aop = tc.alloc_tile_pool(name="aop", bufs=2, side="right")
    rp = tc.alloc_tile_pool(name="rp", bufs=2)
    pexp = tc.alloc_tile_pool(name="pexp", bufs=3)
    evac = tc.alloc_tile_pool(name="evac", bufs=2)
    aogp = tc.alloc_tile_pool(name="aogp", bufs=2, side="right")
    dram = tc.alloc_tile_pool(name="dram", bufs=1, space="DRAM")

    psM = tc.alloc_tile_pool(name="psM", bufs=2, space="PSUM")
    psS = tc.alloc_tile_pool(name="psS", bufs=2, space="PSUM")
    psO = tc.alloc_tile_pool(name="psO", bufs=1, space="PSUM")

    # ------- constants + weights; DMA order tuned for fast start -------
    cos2_t = const.tile([128, S], MMD)
    nc.sync.dma_start(cos2_t[:], cos2[:])
    sin2s_t = const.tile([128, S], MMD)
    nc.sync.dma_start(sin2s_t[:], sin2s[:])
    ident = const.tile([128, 64], F32)
    nc.gpsimd.memset(ident[:], 0.0)
    for p in (0, 64):
        nc.gpsimd.affine_select(
            out=ident[p:p + 64, :], in_=ident[p:p + 64, :],
            compare_op=mybir.AluOpType.not_equal,
            fill=1.0, base=0, pattern=[[-1, 64]], channel_multiplier=1,
        )
    # causal keep-mask for the diagonal blocks (1 where q >= k)
    trimask_t = const.tile([KTILE, KTILE], MMD)
    nc.gpsimd.memset(trimask_t[:], 1.0)
    nc.gpsimd.affine_select(
        out=trimask_t[:], in_=trimask_t[:],
        compare_op=mybir.AluOpType.is_ge,
        fill=0.0, base=0, pattern=[[1, KTILE]], channel_multiplier=-1,
    )

    # x slab for (b=0, j=0) interleaved with the projection weights, spread
    # over both HW DMA queues so the first matmul group starts within ~5us
    xts00 = []
    WQ = {}
    WKV = {}
    for k in range(ND):
        q1, q2 = (nc.sync, nc.scalar) if k % 2 == 0 else (nc.scalar, nc.sync)
        xt = xpool.tile([128, QTILE], MMD, name="xt", tag=f"xt{k}")
        q1.dma_start(xt[:], xT[k * 128:(k + 1) * 128, 0:QTILE])
        xts00.append(xt)
        for t in range(2):
            w = wpool.tile([128, 128], MMD, name=f"wq{t}_{k}")
            q2.dma_start(w[:], wq[k * 128:(k + 1) * 128,
                                  t * 128:(t + 1) * 128])
            WQ[t, k] = w
        w = wpool.tile([128, 128], MMD, name=f"wkv{k}")
        q1.dma_start(w[:], wkv[k * 128:(k + 1) * 128, :])
        WKV[k] = w

    sel_t = const.tile([2 * QH, 4 * KTILE], MMD)
    nc.sync.dma_start(sel_t[:], sel[:])

    # full wo (loaded via the scalar DMA queue; scalar is idle early on)
    WO = {}
    for fc in range(ND):
        for dn in range(4):
            w = wpool.tile([128, QTILE], MMD, name=f"wo{fc}_{dn}")
            nc.scalar.dma_start(
                w[:], wo[fc * 128:(fc + 1) * 128,
                         dn * QTILE:(dn + 1) * QTILE])
            WO[fc, dn] = w

    # persistent K/V cache tiles
    KK = [kvp.tile([128, S], MMD, name=f"kk{b}") for b in range(B)]
    VA = {}
    for b in range(B):
        for i in range(S // KTILE):
            VA[b, i] = kvp.tile([128, HD + 1], MMD, name=f"va{b}_{i}")

    # A2A dram tiles (one pair per slab)
    a2a_in = [dram.tile([FQ * NCORES, KTILE], MMD, name=f"ain{j}")
              for j in range(NSLAB)]
    a2a_out = [dram.tile([FQ * NCORES, KTILE], MMD, name=f"aout{j}")
               for j in range(NSLAB)]

    AO = {}   # per (b, t) slab-local attention output, feature-major
    QT = {}

    def rope(dst, rows, qs, tab_qs):
        # dst[rows, qs] = dst*cos + swap32(dst)*sin  (feature-major RoPE);
        # qs indexes dst columns, tab_qs the (global-position) rope tables
        n = rows[1] - rows[0]
        qsw = rp.tile([128, QTILE], MMD, name="qsw", tag="qsw")
        for p in range(rows[0], rows[1], 64):
            q0 = p - rows[0]
            nc.sync.dma_start(qsw[q0:q0 + 32, :], dst[p + 32:p + 64, qs])
            nc.sync.dma_start(qsw[q0 + 32:q0 + 64, :], dst[p:p + 32, qs])
        t1 = rp.tile([128, QTILE], F32, name="t1", tag="t1")
        nc.vector.tensor_mul(t1[:n], dst[rows[0]:rows[1], qs],
                             cos2_t[rows[0]:rows[1], tab_qs])
        t2 = rp.tile([128, QTILE], F32, name="t2", tag="t2")
        nc.vector.tensor_mul(t2[:n], qsw[:n], sin2s_t[rows[0]:rows[1], tab_qs])
        nc.vector.tensor_add(dst[rows[0]:rows[1], qs], t1[:n], t2[:n])

    def proj(b, j, xts=None):
        qs = slice(j * QTILE, (j + 1) * QTILE)
        if xts is None:
            xts = []
            for k in range(ND):
                xt = xpool.tile([128, QTILE], MMD, name="xt", tag=f"xt{k}")
                nc.sync.dma_start(
                    xt[:], xT[b * D + k * 128:b * D + (k + 1) * 128, qs])
                xts.append(xt)
        for t in range(2):
            ps = psM.tile([128, QTILE], F32, name="psq", tag="mm")
            for k in range(ND):
                nc.tensor.matmul(ps[:], WQ[t, k][:], xts[k][:],
                                 start=(k == 0), stop=(k == ND - 1))
            qt = qpool.tile([128, QTILE], MMD, name="qt", tag=f"qt{b}_{t}")
            QT[b, t] = qt
            nc.vector.tensor_copy(qt[:], ps[:])
            rope(qt, (0, 128), slice(0, QTILE), qs)
        ps = psM.tile([128, QTILE], F32, name="pskv", tag="mm")
        for k in range(ND):
            nc.tensor.matmul(ps[:], WKV[k][:], xts[k][:],
                             start=(k == 0), stop=(k == ND - 1))
        nc.vector.tensor_copy(KK[b][0:64, qs], ps[0:64, :])
        rope(KK[b], (0, 64), qs, qs)
        # duplicate roped K into rows 64:128 (for the row-tiled score pair)
        nc.sync.dma_start(KK[b][64:128, qs], KK[b][0:64, qs])
        vv = rp.tile([128, QTILE], F32, name="vv", tag="vv")
        nc.vector.tensor_copy(vv[64:128, :], ps[64:128, :])
        for c in range(4):
            i = 4 * j + c
            tp = psM.tile([128, QTILE], F32, name="tp", tag="mm")
            nc.tensor.matmul(tp[:, 0:HD], vv[64:128, c * 128:(c + 1) * 128],
                             ident[64:128, :], is_transpose=True,
                             start=True, stop=True)
            va = VA[b, i]
            nc.vector.tensor_copy(va[:, 0:HD], tp[:, 0:HD])
            nc.vector.memset(va[:, HD:HD + 1], 1.0)

    def attn(b, j):
        nkt = 4 * j + 4
        for t in range(2):
            oA = psO.tile([HD + 1, QTILE], F32, name="oA", tag="oA")
            oB = psO.tile([HD + 1, QTILE], F32, name="oB", tag="oB")
            sabs = {}

            def scores(i):
                r = i - 4 * j
                off = max(r, 0) * KTILE
                ks = slice(i * KTILE, (i + 1) * KTILE)
                sAB = psS.tile([128, 2 * QTILE], F32, name="sAB", tag="sAB")
                nc.tensor.matmul(sAB[:, off:QTILE], KK[b][0:64, ks],
                                 QT[b, t][0:64, off:], start=True, stop=True,
                                 tile_position=(0, 0))
                nc.tensor.matmul(sAB[:, QTILE + off:], KK[b][64:128, ks],
                                 QT[b, t][64:128, off:], start=True, stop=True,
                                 tile_position=(64, 0))
                sabs[i] = sAB

            scores(0)
            for i in range(nkt):
                r = i - 4 * j
                off = max(r, 0) * KTILE
                if i + 1 < nkt:
                    scores(i + 1)
                sAB = sabs.pop(i)
                pAB = pexp.tile([128, 2 * QTILE], MMD, name="pAB", tag="pAB")
                nc.scalar.activation(pAB[:, off:], sAB[:, off:], EXP,
                                     scale=SCALE)
                if r >= 0:
                    for h in (off, QTILE + off):
                        # zero the strictly-upper triangle (causal mask)
                        nc.vector.tensor_mul(pAB[:, h:h + KTILE],
                                             pAB[:, h:h + KTILE],
                                             trimask_t[:])
                nc.tensor.matmul(oA[:, off:], VA[b, i][:], pAB[:, off:QTILE],
                                 start=(i == 0), stop=(i == nkt - 1))
                nc.tensor.matmul(oB[:, off:], VA[b, i][:], pAB[:, QTILE + off:],
                                 start=(i == 0), stop=(i == nkt - 1))
            tA = evac.tile([HD + 1, QTILE], MMD, name="tA", tag="tA")
            tB = evac.tile([HD + 1, QTILE], MMD, name="tB", tag="tB")
            nc.vector.tensor_copy(tA[:], oA[:])
            nc.vector.tensor_copy(tB[:], oB[:])
            ao = AO[b, t]
            nc.sync.dma_start(ao[0:64, :], tA[0:64, :])
            nc.sync.dma_start(ao[64:128, :], tB[0:64, :])
            dn = AO["dn"]
            nc.sync.dma_start(dn[4 * b + t:4 * b + t + 1, :], tA[64:65, :])
            nc.sync.dma_start(dn[4 * b + 2 + t:4 * b + 3 + t, :],
                              tB[64:65, :])

    def finish(j):
        # normalize, build the A2A input, kick the A2A
        dn = AO["dn"]
        dnR = evac.tile([2 * QH, QTILE], F32, name="dnR", tag="dnR")
        nc.vector.reciprocal(dnR[:], dn[:])
        dnRb = evac.tile([2 * QH, QTILE], MMD, name="dnRb", tag="dnRb")
        nc.vector.tensor_copy(dnRb[:], dnR[:])
        for b in range(B):
            for t in range(2):
                bc = psM.tile([128, QTILE], F32, name="bc", tag="mm")
                nc.tensor.matmul(
                    bc[:], sel_t[:, (2 * b + t) * 128:(2 * b + t + 1) * 128],
                    dnRb[:], start=True, stop=True)
                nc.vector.tensor_mul(AO[b, t][:], AO[b, t][:], bc[:])
        for d in range(NCORES):
            bd, g = d // 4, d % 4
            for t in range(2):
                nc.gpsimd.dma_start(
                    a2a_in[j][FQ * d + 128 * t:FQ * d + 128 * (t + 1), :],
                    AO[bd, t][:, g * KTILE:(g + 1) * KTILE])
        if single:
            nc.sync.dma_start(a2a_out[j][:], a2a_in[j][:])
        else:
            nc.gpsimd.collective_compute(
                "AllToAll", mybir.AluOpType.bypass,
                replica_groups=[list(range(NCORES))],
                ins=[a2a_in[j][:]], outs=[a2a_out[j][:]],
            )

    def wo_slab(j):
        aogs = []
        for fc in range(ND):
            aog = aogp.tile([128, KTILE], MMD, name="aog", tag=f"aog{fc}")
            nc.sync.dma_start(aog[:], a2a_out[j][fc * 128:(fc + 1) * 128, :])
            aogs.append(aog)
        for dn in range(4):
            psW = psM.tile([128, QTILE], F32, name="psW", tag="mm")
            for fc in range(ND):
                nc.tensor.matmul(psW[:], aogs[fc][:], WO[fc, dn][:],
                                 start=(fc == 0), stop=(fc == ND - 1))
            og = evac.tile([128, QTILE], F32, name="og", tag="og")
            nc.vector.tensor_copy(og[:], psW[:])
            nc.gpsimd.dma_start(
                out_full[j * 128:(j + 1) * 128, dn * QTILE:(dn + 1) * QTILE],
                og[:])

    for j in range(NSLAB):
        AO["dn"] = aop.tile([2 * QH, QTILE], MMD, name="dn", tag="dn")
        for b in range(B):
            AO[b, 0] = aop.tile([128, QTILE], MMD, name=f"ao{b}0",
                                tag=f"ao{b}0")
            AO[b, 1] = aop.tile([128, QTILE], MMD, name=f"ao{b}1",
                                tag=f"ao{b}1")
            proj(b, j, xts00 if (b == 0 and j == 0) else None)
            attn(b, j)
        finish(j)
        if j > 0:
            wo_slab(j - 1)
    wo_slab(NSLAB - 1)

    for p in (psO, psS, psM, dram, aogp, evac, pexp, rp, aop, qpool, xpool,
              kvp, wpool, const):
        p.release()


def _build(single=False):
    nc = bacc.Bacc("TRN2", target_bir_lowering=False, debug=False,
                   num_devices=1 if single else NCORES)
    io = {
        "xT": nc.dram_tensor("xT", [B * D, S], BF16, kind="ExternalInput").ap(),
        "wq": nc.dram_tensor("wq", [D, FQ], BF16, kind="ExternalInput").ap(),
        "wkv": nc.dram_tensor("wkv", [D, FKV], BF16, kind="ExternalInput").ap(),
        "wo": nc.dram_tensor("wo", [D, D], BF16, kind="ExternalInput").ap(),
        "cos2": nc.dram_tensor("cos2", [128, S], BF16, kind="ExternalInput").ap(),
        "sin2s": nc.dram_tensor("sin2s", [128, S], BF16, kind="ExternalInput").ap(),
        "sel": nc.dram_tensor("sel", [2 * QH, 4 * KTILE], BF16,
                              kind="ExternalInput").ap(),
        "out": nc.dram_tensor("out", [NSLAB * 128, D], F32,
                              kind="ExternalOutput").ap(),
    }
    io["single"] = single
    with tile.TileContext(nc) as tc:
        _build_kernel(tc, io)
    nc.compile()
    return nc


_CACHE = {}


def _get_program():
    if "nc" not in _CACHE:
        _CACHE["nc"] = _build()
    return _CACHE["nc"]


def _host_inputs(x, wq, wk, wv, wo):
    x = np.ascontiguousarray(x, np.float32)
    inv = 1.0 / (10000.0 ** (np.arange(0, HD, 2, dtype=np.float64) / HD))
    pos = np.arange(S, dtype=np.float64)
    freqs = np.outer(pos, inv)                      # [S, 32]
    emb = np.concatenate([freqs, freqs], axis=1)    # [S, 64]
    cos = np.cos(emb).T.astype(np.float32)          # [64, S]
    sin = np.sin(emb).T.astype(np.float32)
    cos2 = np.concatenate([cos, cos], axis=0)       # [128, S]
    sin2s = np.concatenate([-sin[:32], sin[32:], -sin[:32], sin[32:]], axis=0)

    # denominator broadcast selector: for (b, t) block, AO[b,t] rows 0:64
    # <- dn row 4b+t, rows 64:128 <- dn row 4b+2+t
    sel = np.zeros((2 * QH, 4 * KTILE), np.float32)
    for b in range(2):
        for t in range(2):
            blk = (2 * b + t) * 128
            sel[4 * b + t, blk:blk + 64] = 1.0
            sel[4 * b + 2 + t, blk + 64:blk + 128] = 1.0

    import ml_dtypes
    bf16 = ml_dtypes.bfloat16
    cos2 = cos2.astype(bf16)
    sin2s = sin2s.astype(bf16)
    sel = sel.astype(bf16)
    xT = np.ascontiguousarray(
        np.concatenate([x[0].T, x[1].T], axis=0).astype(bf16))  # [2D, S]

    # wo rows ordered to match the gathered A2A feature order:
    # src core cc contributes heads (4cc+t, 4cc+t+2) for t in (0, 1)
    wrows = []
    for cc in range(NCORES):
        for t in range(2):
            for h in (4 * cc + t, 4 * cc + t + 2):
                wrows.append(wo[h * HD:(h + 1) * HD, :])
    wo_p = np.ascontiguousarray(np.concatenate(wrows, axis=0).astype(bf16))

    in_maps = []
    for c in range(NCORES):
        qcols = []
        for t in range(2):
            for h in (4 * c + t, 4 * c + t + 2):
                qcols.append(wq[:, h * HD:(h + 1) * HD])
        wq_p = np.ascontiguousarray(np.concatenate(qcols, axis=1).astype(bf16))
        wkv_p = np.ascontiguousarray(np.concatenate(
            [wk[:, c * HD:(c + 1) * HD], wv[:, c * HD:(c + 1) * HD]],
            axis=1).astype(bf16))
        in_maps.append({
            "xT": xT, "wq": wq_p, "wkv": wkv_p, "wo": wo_p,
            "cos2": cos2, "sin2s": sin2s, "sel": sel,
        })
    return in_maps


def run(x, wq, wk, wv, wo, trace=False, **trace_kwargs):
    nc = _get_program()
    in_maps = _host_inputs(x, wq, wk, wv, wo)
    res = run_bass_kernel_spmd(nc, in_maps, list(range(NCORES)),
                               trace=trace, **trace_kwargs)
    out = np.empty((B, S, D), np.float32)
    for c in range(NCORES):
        bo, g = c // 4, c % 4
        shard = res.results[c]["out"]  # [512, D]
        for j in range(NSLAB):
            out[bo, j * QTILE + g * 128:j * QTILE + (g + 1) * 128, :] = \
                shard[j * 128:(j + 1) * 128, :]
    return out, res


def kernel(x, wq, wk, wv, wo):
    out, _ = run(x, wq, wk, wv, wo)
    return out.astype(np.float32)


# revision 27
# speedup vs baseline: 1.3899x; 1.0051x over previous
"""Tensor-parallel GQA multi-head attention (RoPE + causal softmax) for 8 trn2 cores.

Sharding v2: every core handles BOTH batches with 4 q-heads / 1 kv-head:
core c owns q-heads {4c..4c+3} (kv-head c) of batches 0 and 1. Attention
runs in transposed (feature-major) layout with flash-style causal tiling.
Per 512-token slab, the 8 cores exchange their normalized attention outputs
with one AllToAll (bf16, 512KB) so that core c ends up with ALL 2048
attention features for its 128-position output stripe (batch c//4, stripe
c%4); it then applies the full wo to produce disjoint output rows. No
reduction collective is needed.
"""

import sys

sys.path.insert(0, "/opt/trn_rl_repo")

import numpy as np

import concourse.bass as bass
import concourse.bacc as bacc
import concourse.mybir as mybir
from concourse import tile
from concourse.bass_utils import run_bass_kernel_spmd

B, S, D = 2, 2048, 2048
N_HEADS, N_KV, HD = 32, 8, 64
NCORES = 8
QH = 4    # q-heads per core
FQ = QH * HD       # 256 q-feature cols per core
FKV = 2 * HD       # 128 (K then V) per core
SCALE = 1.0 / 8.0  # 1/sqrt(HD)

QTILE = 512
KTILE = 128
NSLAB = S // QTILE  # 4
ND = D // 128       # 16 contraction chunks

F32 = mybir.dt.float32
EXP = mybir.ActivationFunctionType.Exp
BF16 = mybir.dt.bfloat16
MMD = BF16
LE = mybir.AluOpType.is_ge


def _build_kernel(tc, io):
    nc = tc.nc
    xT, wq, wkv, wo = io["xT"], io["wq"], io["wkv"], io["wo"]
    cos2, sin2s, sel = io["cos2"], io["sin2s"], io["sel"]
    out_full = io["out"]
    single = bool(io.get("single"))

    # ---------------- pools ----------------
    const = tc.alloc_tile_pool(name="const", bufs=1)
    wpool = tc.alloc_tile_pool(name="wpool", bufs=1, side="right")
    kvp = tc.alloc_tile_pool(name="kvp", bufs=1)
    xpool = tc.alloc_tile_pool(name="xpool", bufs=2)
    qpool = tc.alloc_tile_pool(name="qpool", bufs=2)
    aop = tc.alloc_tile_pool(name="aop", bufs=2, side="right")
    rp = tc.alloc_tile_pool(name="rp", bufs=2)
    pexp = tc.alloc_tile_pool(name="pexp", bufs=3)
    evac = tc.alloc_tile_pool(name="evac", bufs=2)
    aogp = tc.alloc_tile_pool(name="aogp", bufs=2, side="right")
    dram = tc.alloc_tile_pool(name="dram", bufs=1, space="DRAM")

    psM = tc.alloc_tile_pool(name="psM", bufs=2, space="PSUM")
    psS = tc.alloc_tile_pool(name="psS", bufs=2, space="PSUM")
    psO = tc.alloc_tile_pool(name="psO", bufs=1, space="PSUM")

    # ------- constants + weights; DMA order tuned for fast start -------
    cos2_t = const.tile([128, S], MMD)
    nc.sync.dma_start(cos2_t[:], cos2[:])
    sin2s_t = const.tile([128, S], MMD)
    nc.sync.dma_start(sin2s_t[:], sin2s[:])
    ident = const.tile([128, 64], F32)
    nc.gpsimd.memset(ident[:], 0.0)
    for p in (0, 64):
        nc.gpsimd.affine_select(
            out=ident[p:p + 64, :], in_=ident[p:p + 64, :],
            compare_op=mybir.AluOpType.not_equal,
            fill=1.0, base=0, pattern=[[-1, 64]], channel_multiplier=1,
        )
    # causal keep-mask for the diagonal blocks (1 where q >= k)
    trimask_t = const.tile([KTILE, KTILE], MMD)
    nc.gpsimd.memset(trimask_t[:], 1.0)
    nc.gpsimd.affine_select(
        out=trimask_t[:], in_=trimask_t[:],
        compare_op=mybir.AluOpType.is_ge,
        fill=0.0, base=0, pattern=[[1, KTILE]], channel_multiplier=-1,
    )

    # x slab for (b=0, j=0) interleaved with the projection weights, spread
    # over both HW DMA queues so the first matmul group starts within ~5us
    xts00 = []
    WQ = {}
    WKV = {}
    for k in range(ND):
        q1, q2 = (nc.sync, nc.scalar) if k % 2 == 0 else (nc.scalar, nc.sync)
        xt = xpool.tile([128, QTILE], MMD, name="xt", tag=f"xt{k}")
        q1.dma_start(xt[:], xT[k * 128:(k + 1) * 128, 0:QTILE])
        xts00.append(xt)
        for t in range(2):
            w = wpool.tile([128, 128], MMD, name=f"wq{t}_{k}")
            q2.dma_start(w[:], wq[k * 128:(k + 1) * 128,
                                  t * 128:(t + 1) * 128])
            WQ[t, k] = w
        w = wpool.tile([128, 128], MMD, name=f"wkv{k}")
        q1.dma_start(w[:], wkv[k * 128:(k + 1) * 128, :])
        WKV[k] = w

    sel_t = const.tile([2 * QH, 4 * KTILE], MMD)
    nc.sync.dma_start(sel_t[:], sel[:])

    # full wo (loaded via the scalar DMA queue; scalar is idle early on)
    WO = {}
    for fc in range(ND):
        for dn in range(4):
            w = wpool.tile([128, QTILE], MMD, name=f"wo{fc}_{dn}")
            nc.scalar.dma_start(
                w[:], wo[fc * 128:(fc + 1) * 128,
                         dn * QTILE:(dn + 1) * QTILE])
            WO[fc, dn] = w

    # persistent K/V cache tiles
    KK = [kvp.tile([128, S], MMD, name=f"kk{b}") for b in range(B)]
    VA = {}
    for b in range(B):
        for i in range(S // KTILE):
            VA[b, i] = kvp.tile([128, HD + 1], MMD, name=f"va{b}_{i}")

    # A2A dram tiles (one pair per slab)
    a2a_in = [dram.tile([FQ * NCORES, KTILE], MMD, name=f"ain{j}")
              for j in range(NSLAB)]
    a2a_out = [dram.tile([FQ * NCORES, KTILE], MMD, name=f"aout{j}")
               for j in range(NSLAB)]

    AO = {}   # per (b, t) slab-local attention output, feature-major
    QT = {}

    def rope(dst, rows, qs, tab_qs):
        # dst[rows, qs] = dst*cos + swap32(dst)*sin  (feature-major RoPE);
        # qs indexes dst columns, tab_qs the (global-position) rope tables
        n = rows[1] - rows[0]
        qsw = rp.tile([128, QTILE], MMD, name="qsw", tag="qsw")
        for p in range(rows[0], rows[1], 64):
            q0 = p - rows[0]
            nc.sync.dma_start(qsw[q0:q0 + 32, :], dst[p + 32:p + 64, qs])
            nc.sync.dma_start(qsw[q0 + 32:q0 + 64, :], dst[p:p + 32, qs])
        t1 = rp.tile([128, QTILE], F32, name="t1", tag="t1")
        nc.vector.tensor_mul(t1[:n], dst[rows[0]:rows[1], qs],
                             cos2_t[rows[0]:rows[1], tab_qs])
        t2 = rp.tile([128, QTILE], F32, name="t2", tag="t2")
        nc.vector.tensor_mul(t2[:n], qsw[:n], sin2s_t[rows[0]:rows[1], tab_qs])
        nc.vector.tensor_add(dst[rows[0]:rows[1], qs], t1[:n], t2[:n])

    def prefetch_x(b, j):
        qs = slice(j * QTILE, (j + 1) * QTILE)
        xts = []
        for k in range(ND):
            xt = xpool.tile([128, QTILE], MMD, name="xt", tag=f"xt{k}")
            nc.sync.dma_start(
                xt[:], xT[b * D + k * 128:b * D + (k + 1) * 128, qs])
            xts.append(xt)
        return xts

    def proj(b, j, xts=None):
        qs = slice(j * QTILE, (j + 1) * QTILE)
        if xts is None:
            xts = prefetch_x(b, j)
        for t in range(2):
            ps = psM.tile([128, QTILE], F32, name="psq", tag="mm")
            for k in range(ND):
                nc.tensor.matmul(ps[:], WQ[t, k][:], xts[k][:],
                                 start=(k == 0), stop=(k == ND - 1))
            qt = qpool.tile([128, QTILE], MMD, name="qt", tag=f"qt{b}_{t}")
            QT[b, t] = qt
            nc.vector.tensor_copy(qt[:], ps[:])
            rope(qt, (0, 128), slice(0, QTILE), qs)
        ps = psM.tile([128, QTILE], F32, name="pskv", tag="mm")
        for k in range(ND):
            nc.tensor.matmul(ps[:], WKV[k][:], xts[k][:],
                             start=(k == 0), stop=(k == ND - 1))
        nc.vector.tensor_copy(KK[b][0:64, qs], ps[0:64, :])
        rope(KK[b], (0, 64), qs, qs)
        # duplicate roped K into rows 64:128 (for the row-tiled score pair)
        nc.sync.dma_start(KK[b][64:128, qs], KK[b][0:64, qs])
        vv = rp.tile([128, QTILE], F32, name="vv", tag="vv")
        nc.vector.tensor_copy(vv[64:128, :], ps[64:128, :])
        for c in range(4):
            i = 4 * j + c
            tp = psM.tile([128, QTILE], F32, name="tp", tag="mm")
            nc.tensor.matmul(tp[:, 0:HD], vv[64:128, c * 128:(c + 1) * 128],
                             ident[64:128, :], is_transpose=True,
                             start=True, stop=True)
            va = VA[b, i]
            nc.vector.tensor_copy(va[:, 0:HD], tp[:, 0:HD])
            nc.vector.memset(va[:, HD:HD + 1], 1.0)

    def attn(b, j, fillers=None):
        # fillers: closures emitting small independent PE chunks (wo matmuls
        # of the previous slab); placed before each attnV so the tensor
        # engine has work while it would otherwise stall on the exp
        fillers = list(fillers or [])
        nkt = 4 * j + 4
        for t in range(2):
            oA = psO.tile([HD + 1, QTILE], F32, name="oA", tag="oA")
            oB = psO.tile([HD + 1, QTILE], F32, name="oB", tag="oB")
            sabs = {}

            def scores(i):
                r = i - 4 * j
                off = max(r, 0) * KTILE
                ks = slice(i * KTILE, (i + 1) * KTILE)
                sAB = psS.tile([128, 2 * QTILE], F32, name="sAB", tag="sAB")
                nc.tensor.matmul(sAB[:, off:QTILE], KK[b][0:64, ks],
                                 QT[b, t][0:64, off:], start=True, stop=True,
                                 tile_position=(0, 0))
                nc.tensor.matmul(sAB[:, QTILE + off:], KK[b][64:128, ks],
                                 QT[b, t][64:128, off:], start=True, stop=True,
                                 tile_position=(64, 0))
                sabs[i] = sAB

            scores(0)
            for i in range(nkt):
                r = i - 4 * j
                off = max(r, 0) * KTILE
                if i + 1 < nkt:
                    scores(i + 1)
                sAB = sabs.pop(i)
                pAB = pexp.tile([128, 2 * QTILE], MMD, name="pAB", tag="pAB")
                nc.scalar.activation(pAB[:, off:], sAB[:, off:], EXP,
                                     scale=SCALE)
                if r >= 0:
                    for h in (off, QTILE + off):
                        # zero the strictly-upper triangle (causal mask)
                        nc.vector.tensor_mul(pAB[:, h:h + KTILE],
                                             pAB[:, h:h + KTILE],
                                             trimask_t[:])
                if fillers:
                    fillers.pop(0)()
                nc.tensor.matmul(oA[:, off:], VA[b, i][:], pAB[:, off:QTILE],
                                 start=(i == 0), stop=(i == nkt - 1))
                nc.tensor.matmul(oB[:, off:], VA[b, i][:], pAB[:, QTILE + off:],
                                 start=(i == 0), stop=(i == nkt - 1))
            tA = evac.tile([HD + 1, QTILE], MMD, name="tA", tag="tA")
            tB = evac.tile([HD + 1, QTILE], MMD, name="tB", tag="tB")
            nc.vector.tensor_copy(tA[:], oA[:])
            nc.vector.tensor_copy(tB[:], oB[:])
            ao = AO[b, t]
            nc.sync.dma_start(ao[0:64, :], tA[0:64, :])
            nc.sync.dma_start(ao[64:128, :], tB[0:64, :])
            dn = AO["dn"]
            nc.sync.dma_start(dn[4 * b + t:4 * b + t + 1, :], tA[64:65, :])
            nc.sync.dma_start(dn[4 * b + 2 + t:4 * b + 3 + t, :],
                              tB[64:65, :])
        for f in fillers:
            f()

    def finish(j):
        # normalize, build the A2A input, kick the A2A
        dn = AO["dn"]
        dnR = evac.tile([2 * QH, QTILE], F32, name="dnR", tag="dnR")
        nc.vector.reciprocal(dnR[:], dn[:])
        dnRb = evac.tile([2 * QH, QTILE], MMD, name="dnRb", tag="dnRb")
        nc.vector.tensor_copy(dnRb[:], dnR[:])
        for b in range(B):
            for t in range(2):
                bc = psM.tile([128, QTILE], F32, name="bc", tag="mm")
                nc.tensor.matmul(
                    bc[:], sel_t[:, (2 * b + t) * 128:(2 * b + t + 1) * 128],
                    dnRb[:], start=True, stop=True)
                nc.vector.tensor_mul(AO[b, t][:], AO[b, t][:], bc[:])
        for d in range(NCORES):
            bd, g = d // 4, d % 4
            for t in range(2):
                nc.gpsimd.dma_start(
                    a2a_in[j][FQ * d + 128 * t:FQ * d + 128 * (t + 1), :],
                    AO[bd, t][:, g * KTILE:(g + 1) * KTILE])
        if single:
            nc.sync.dma_start(a2a_out[j][:], a2a_in[j][:])
        else:
            nc.gpsimd.collective_compute(
                "AllToAll", mybir.AluOpType.bypass,
                replica_groups=[list(range(NCORES))],
                ins=[a2a_in[j][:]], outs=[a2a_out[j][:]],
            )

    def make_wo_fillers(j):
        # wo for slab j, split into one gather step plus 4x4 matmul chunks
        ctx = {}

        def gather():
            ctx["aogs"] = []
            for fc in range(ND):
                aog = aogp.tile([128, KTILE], MMD, name="aog", tag=f"aog{fc}")
                nc.sync.dma_start(aog[:],
                                  a2a_out[j][fc * 128:(fc + 1) * 128, :])
                ctx["aogs"].append(aog)

        fillers = [gather]
        for dn in range(4):
            for sub in range(4):
                def chunk(dn=dn, sub=sub):
                    if sub == 0:
                        ctx[dn] = psM.tile([128, QTILE], F32, name="psW",
                                           tag="mm")
                    ps = ctx[dn]
                    for fc in range(4 * sub, 4 * sub + 4):
                        nc.tensor.matmul(ps[:], ctx["aogs"][fc][:],
                                         WO[fc, dn][:],
                                         start=(fc == 0), stop=(fc == ND - 1))
                    if sub == 3:
                        og = evac.tile([128, QTILE], F32, name="og", tag="og")
                        nc.vector.tensor_copy(og[:], ps[:])
                        nc.gpsimd.dma_start(
                            out_full[j * 128:(j + 1) * 128,
                                     dn * QTILE:(dn + 1) * QTILE], og[:])
                fillers.append(chunk)
        return fillers

    def wo_slab(j):
        for f in make_wo_fillers(j):
            f()

    nxt = {0: xts00, 1: None}
    for j in range(NSLAB):
        AO["dn"] = aop.tile([2 * QH, QTILE], MMD, name="dn", tag="dn")
        for b in range(B):
            AO[b, 0] = aop.tile([128, QTILE], MMD, name=f"ao{b}0",
                                tag=f"ao{b}0")
            AO[b, 1] = aop.tile([128, QTILE], MMD, name=f"ao{b}1",
                                tag=f"ao{b}1")
        proj(0, j, nxt[0])
        attn(0, j)
        nxt[0] = prefetch_x(0, j + 1) if j + 1 < NSLAB else None
        proj(1, j, nxt[1])
        attn(1, j, fillers=make_wo_fillers(j - 1) if j > 0 else None)
        nxt[1] = prefetch_x(1, j + 1) if j + 1 < NSLAB else None
        finish(j)
    wo_slab(NSLAB - 1)

    for p in (psO, psS, psM, dram, aogp, evac, pexp, rp, aop, qpool, xpool,
              kvp, wpool, const):
        p.release()


def _build(single=False):
    nc = bacc.Bacc("TRN2", target_bir_lowering=False, debug=False,
                   num_devices=1 if single else NCORES)
    io = {
        "xT": nc.dram_tensor("xT", [B * D, S], BF16, kind="ExternalInput").ap(),
        "wq": nc.dram_tensor("wq", [D, FQ], BF16, kind="ExternalInput").ap(),
        "wkv": nc.dram_tensor("wkv", [D, FKV], BF16, kind="ExternalInput").ap(),
        "wo": nc.dram_tensor("wo", [D, D], BF16, kind="ExternalInput").ap(),
        "cos2": nc.dram_tensor("cos2", [128, S], BF16, kind="ExternalInput").ap(),
        "sin2s": nc.dram_tensor("sin2s", [128, S], BF16, kind="ExternalInput").ap(),
        "sel": nc.dram_tensor("sel", [2 * QH, 4 * KTILE], BF16,
                              kind="ExternalInput").ap(),
        "out": nc.dram_tensor("out", [NSLAB * 128, D], F32,
                              kind="ExternalOutput").ap(),
    }
    io["single"] = single
    with tile.TileContext(nc) as tc:
        _build_kernel(tc, io)
    nc.compile()
    return nc


_CACHE = {}


def _get_program():
    if "nc" not in _CACHE:
        _CACHE["nc"] = _build()
    return _CACHE["nc"]


def _host_inputs(x, wq, wk, wv, wo):
    x = np.ascontiguousarray(x, np.float32)
    inv = 1.0 / (10000.0 ** (np.arange(0, HD, 2, dtype=np.float64) / HD))
    pos = np.arange(S, dtype=np.float64)
    freqs = np.outer(pos, inv)                      # [S, 32]
    emb = np.concatenate([freqs, freqs], axis=1)    # [S, 64]
    cos = np.cos(emb).T.astype(np.float32)          # [64, S]
    sin = np.sin(emb).T.astype(np.float32)
    cos2 = np.concatenate([cos, cos], axis=0)       # [128, S]
    sin2s = np.concatenate([-sin[:32], sin[32:], -sin[:32], sin[32:]], axis=0)

    # denominator broadcast selector: for (b, t) block, AO[b,t] rows 0:64
    # <- dn row 4b+t, rows 64:128 <- dn row 4b+2+t
    sel = np.zeros((2 * QH, 4 * KTILE), np.float32)
    for b in range(2):
        for t in range(2):
            blk = (2 * b + t) * 128
            sel[4 * b + t, blk:blk + 64] = 1.0
            sel[4 * b + 2 + t, blk + 64:blk + 128] = 1.0

    import ml_dtypes
    bf16 = ml_dtypes.bfloat16
    cos2 = cos2.astype(bf16)
    sin2s = sin2s.astype(bf16)
    sel = sel.astype(bf16)
    xT = np.ascontiguousarray(
        np.concatenate([x[0].T, x[1].T], axis=0).astype(bf16))  # [2D, S]

    # wo rows ordered to match the gathered A2A feature order:
    # src core cc contributes heads (4cc+t, 4cc+t+2) for t in (0, 1)
    wrows = []
    for cc in range(NCORES):
        for t in range(2):
            for h in (4 * cc + t, 4 * cc + t + 2):
                wrows.append(wo[h * HD:(h + 1) * HD, :])
    wo_p = np.ascontiguousarray(np.concatenate(wrows, axis=0).astype(bf16))

    in_maps = []
    for c in range(NCORES):
        qcols = []
        for t in range(2):
            for h in (4 * c + t, 4 * c + t + 2):
                qcols.append(wq[:, h * HD:(h + 1) * HD])
        wq_p = np.ascontiguousarray(np.concatenate(qcols, axis=1).astype(bf16))
        wkv_p = np.ascontiguousarray(np.concatenate(
            [wk[:, c * HD:(c + 1) * HD], wv[:, c * HD:(c + 1) * HD]],
            axis=1).astype(bf16))
        in_maps.append({
            "xT": xT, "wq": wq_p, "wkv": wkv_p, "wo": wo_p,
            "cos2": cos2, "sin2s": sin2s, "sel": sel,
        })
    return in_maps


def run(x, wq, wk, wv, wo, trace=False, **trace_kwargs):
    nc = _get_program()
    in_maps = _host_inputs(x, wq, wk, wv, wo)
    res = run_bass_kernel_spmd(nc, in_maps, list(range(NCORES)),
                               trace=trace, **trace_kwargs)
    out = np.empty((B, S, D), np.float32)
    for c in range(NCORES):
        bo, g = c // 4, c % 4
        shard = res.results[c]["out"]  # [512, D]
        for j in range(NSLAB):
            out[bo, j * QTILE + g * 128:j * QTILE + (g + 1) * 128, :] = \
                shard[j * 128:(j + 1) * 128, :]
    return out, res


def kernel(x, wq, wk, wv, wo):
    out, _ = run(x, wq, wk, wv, wo)
    return out.astype(np.float32)
